# revision 15
# baseline (speedup 1.0000x reference)
"""BERT self-attention (B=8, S=1024, D=1024, H=16, DH=64) on 8 Trainium2 cores.

Strategy: pure data-parallel over batch - each of the 8 cores runs the full
self-attention for one batch element. No collectives.

Per-core kernel layout (S=seq, D=model, H=heads, DH=64):
  - X^T built once via PE transposes (fp32, 64 tiles of 128x128).
  - Q^T[j,s], K^T[j,s] computed directly in transposed orientation
    (contraction over d_in on partitions); biases folded in as K=1 rank-1
    matmuls (b x ones).  Each weight tile is double-pumped over both 512-col
    halves of a [128,1024] PSUM tile (consecutive same-weight matmuls skip
    the serial weight reload - measured 2.2x faster).
  - V[s,j] in natural orientation (lhsT = X^T as weights), stored bf16 in a
    head-interleaved layout of 65-column blocks: [64 V cols | ones col] per
    head.  The ones column makes the context matmul emit the softmax
    denominator for free.
  - scores computed TRANSPOSED: S^T[k,q], so the attention mask (indexed by
    k) is a per-partition bias folded with the 1/sqrt(DH) scale into the Exp
    activation: P^T = exp(scale*S^T + mask[k]), output bf16.
  - context: ctx[q,0:64] + rowsum at col 64 via lhsT=P^T tile (bf16),
    rhs = V' block [128,65]; normalize with vector reciprocal +
    per-partition tensor_scalar multiply, DMA straight to DRAM.
  - attention is software-pipelined by one head: PE runs ctx(h-1) while ACT
    runs exp(h), keeping both engines busy.
  - matmul dtypes: float32r for projections/scores; bf16 for probs@V.

Built on bacc.Bacc: its compile() legalizes sync waits (1 wait/instruction
hardware limit) via move_matmul_waits_to_ldweights + generate_event_semaphores.
"""

import numpy as np

import concourse.bass as bass
import concourse.bacc as bacc
import concourse.mybir as mybir
import concourse.tile as tile
from concourse.bass_utils import run_bass_kernel_spmd
from concourse.masks import make_identity

F32 = mybir.dt.float32
F32R = mybir.dt.float32r
BF16 = mybir.dt.bfloat16

B, S, D, H = 8, 1024, 1024, 16
DH = D // H  # 64
P = 128
NT = S // P  # 8 tiles along any 1024 dim
SC = S // 512  # 2 chunks of 512
SCALE = 1.0 / float(np.sqrt(DH))
N_CORES = 8
VW = DH + 1  # 65: V block width per head (64 cols + ones col)

PHASES = 7  # bitmask: 1=x^T, 2=projections, 4=attention (profiling aid)


def emit_body(nc, dram, pools):
    (x_d, m_d, wq_d, bq_d, wk_d, bk_d, wv_d, bv_d, o_d) = dram
    (cst, xT_pool, qT_pool, kT_pool, v_pool, wx_pool, p_pool, small_pool,
     ps_t, ps_big, ps_ctx, ident) = pools

    # ---- per-body constants (mask / bias rows) ----
    mask_cols = cst.tile([P, NT], F32, name="mask_cols", tag="mask_cols")
    nc.sync.dma_start(out=mask_cols, in_=m_d.ap().rearrange("(g p) -> p g", p=P))
    ones_f32 = cst.tile([1, 512], F32, name="ones_f32", tag="ones_f32")
    nc.vector.memset(ones_f32, 1.0)
    ones_row = cst.tile([1, 512], F32R, name="ones_row", tag="ones_row")
    nc.vector.tensor_copy(ones_row, ones_f32)
    b_rows = {}
    for nm, hd in (("bq", bq_d), ("bk", bk_d), ("bv", bv_d)):
        t = cst.tile([1, D], F32R, name=f"brow_{nm}", tag=f"brow_{nm}")
        nc.sync.dma_start(out=t, in_=hd.ap().unsqueeze(0).bitcast(F32R))
        b_rows[nm] = t

    if not PHASES & 1:
        return
    # ---- phase 1: X^T via PE transposes ----
    xT = []
    for it in range(NT):
        xT.append(xT_pool.tile([P, S], F32R, name=f"xT{it}", tag=f"xT{it}"))
    for st in range(NT):
        x_t = wx_pool.tile([P, D], F32, name="x_tile", tag="wx")
        nc.sync.dma_start(out=x_t, in_=x_d.ap()[st * P : (st + 1) * P, :])
        for it in range(NT):
            pt = ps_t.tile([P, P], F32, name="pt", tag="mm")
            nc.tensor.transpose(pt, x_t[:, it * P : (it + 1) * P], ident)
            nc.vector.tensor_copy(xT[it][:, st * P : (st + 1) * P], pt)

    if not PHASES & 2:
        fin = small_pool.tile([P, DH], F32, name="fin1", tag="bounce")
        nc.vector.tensor_copy(fin, xT[0][:, 0:DH].bitcast(F32))
        nc.sync.dma_start(out=o_d.ap()[0:P, 0:DH], in_=fin)
        return

    # ---- phase 2: projections (double-pumped weights) ----
    def load_w(w_d):
        tiles = []
        for it in range(NT):
            t = wx_pool.tile([P, D], F32R, name="w_tile", tag="wx")
            nc.sync.dma_start(
                out=t, in_=w_d.ap()[it * P : (it + 1) * P, :].bitcast(F32R)
            )
            tiles.append(t)
        return tiles

    # Q^T and K^T: out[j, s] = sum_i W[i, j] * X^T[i, s] + b[j]
    proj_T = {}
    for nm, w_dram, dst_pool in (("bq", wq_d, qT_pool), ("bk", wk_d, kT_pool)):
        w_tiles = load_w(w_dram)
        dst = []
        for jt in range(NT):
            dst.append(
                dst_pool.tile([P, S], F32R, name=f"{nm}T{jt}", tag=f"{nm}T{jt}")
            )
        for jt in range(NT):
            mm = ps_big.tile([P, S], F32, name="mm", tag="big")
            for it in range(NT):
                for sc in range(SC):
                    nc.tensor.matmul(
                        mm[:, sc * 512 : (sc + 1) * 512],
                        lhsT=w_tiles[it][:, jt * P : (jt + 1) * P],
                        rhs=xT[it][:, sc * 512 : (sc + 1) * 512],
                        start=(it == 0),
                        stop=False,
                    )
            for sc in range(SC):
                nc.tensor.matmul(
                    mm[:, sc * 512 : (sc + 1) * 512],
                    lhsT=b_rows[nm][0:1, jt * P : (jt + 1) * P],
                    rhs=ones_row,
                    start=False,
                    stop=True,
                )
            nc.vector.tensor_copy(dst[jt], mm)
        proj_T[nm] = dst
    qT, kT = proj_T["bq"], proj_T["bk"]

    # V: out[s, j] = sum_i X^T[i, s] * Wv[i, j] + bv[j], stored bf16 in
    # 65-wide head blocks with a trailing ones column.
    wv_tiles = load_w(wv_d)
    v_sb = []
    for st in range(NT):
        v = v_pool.tile([P, H * VW], BF16, name=f"v{st}", tag=f"v{st}")
        nc.gpsimd.memset(v, 1.0)  # ones columns survive at h*65+64
        v_sb.append(v)
    for st in range(NT):
        mm = ps_big.tile([P, S], F32, name="mmv", tag="big")
        for it in range(NT):
            for jc in range(SC):
                nc.tensor.matmul(
                    mm[:, jc * 512 : (jc + 1) * 512],
                    lhsT=xT[it][:, st * P : (st + 1) * P],
                    rhs=wv_tiles[it][:, jc * 512 : (jc + 1) * 512],
                    start=(it == 0),
                    stop=False,
                )
        for jc in range(SC):
            nc.tensor.matmul(
                mm[:, jc * 512 : (jc + 1) * 512],
                lhsT=ones_row[0:1, 0:P],
                rhs=b_rows["bv"][0:1, jc * 512 : (jc + 1) * 512],
                start=False,
                stop=True,
            )
        dst = v_sb[st].rearrange("p (g c) -> p g c", c=VW)[:, :, 0:DH]
        src = mm.rearrange("p (g c) -> p g c", c=DH)
        nc.vector.tensor_copy(dst, src)

    if not PHASES & 4:
        fin = small_pool.tile([P, DH], F32, name="fin2", tag="bounce")
        nc.vector.tensor_copy(fin, qT[0][:, 0:DH].bitcast(F32))
        nc.sync.dma_start(out=o_d.ap()[0:P, 0:DH], in_=fin)
        fin2 = small_pool.tile([P, DH], F32, name="fin3", tag="bounce")
        nc.vector.tensor_copy(fin2, kT[0][:, 0:DH].bitcast(F32))
        nc.sync.dma_start(out=o_d.ap()[0:P, DH : 2 * DH], in_=fin2)
        return

    # ---- phase 3: attention, software-pipelined by one head ----
    def emit_scores_exp(h):
        jt, ro = h // 2, (h % 2) * DH
        pT = []
        for kt in range(NT):
            sps = ps_big.tile([P, S], F32, name="sps", tag="big")
            for qc in range(SC):
                nc.tensor.matmul(
                    sps[:, qc * 512 : (qc + 1) * 512],
                    lhsT=kT[jt][ro : ro + DH, kt * P : (kt + 1) * P],
                    rhs=qT[jt][ro : ro + DH, qc * 512 : (qc + 1) * 512],
                    start=True,
                    stop=True,
                )
            pt = p_pool.tile([P, S], BF16, name="pT", tag="pT")
            nc.scalar.activation(
                pt,
                sps,
                mybir.ActivationFunctionType.Exp,
                bias=mask_cols[:, kt : kt + 1],
                scale=SCALE,
            )
            pT.append(pt)
        return pT

    def emit_ctx(h, pT):
        for qt in range(NT):
            cps = ps_ctx.tile([P, VW], F32, name="cps", tag="ctx")
            for kt in range(NT):
                nc.tensor.matmul(
                    cps,
                    lhsT=pT[kt][:, qt * P : (qt + 1) * P],
                    rhs=v_sb[kt][:, h * VW : (h + 1) * VW],
                    start=(kt == 0),
                    stop=(kt == NT - 1),
                )
            r = small_pool.tile([P, 1], F32, name="recip", tag="recip")
            nc.vector.reciprocal(r, cps[:, DH : DH + 1])
            bounce = small_pool.tile([P, DH], F32, name="bounce", tag="bounce")
            nc.vector.tensor_scalar_mul(bounce, cps[:, 0:DH], r)
            nc.sync.dma_start(
                out=o_d.ap()[qt * P : (qt + 1) * P, h * DH : (h + 1) * DH],
                in_=bounce,
            )

    prev = None
    for h in range(H):
        pT = emit_scores_exp(h)
        if prev is not None:
            emit_ctx(h - 1, prev)
        prev = pT
    emit_ctx(H - 1, prev)


def build_program(n_reps: int = 1, n_loop: int = 0) -> bass.Bass:
    nc = bacc.Bacc(trn_type="TRN2", target_bir_lowering=False, debug=False)

    x_d = nc.declare_dram_parameter("hidden_states", [S, D], F32, isOutput=False)
    m_d = nc.declare_dram_parameter("attention_mask", [S], F32, isOutput=False)
    wq_d = nc.declare_dram_parameter("Wq", [D, D], F32, isOutput=False)
    bq_d = nc.declare_dram_parameter("bq", [D], F32, isOutput=False)
    wk_d = nc.declare_dram_parameter("Wk", [D, D], F32, isOutput=False)
    bk_d = nc.declare_dram_parameter("bk", [D], F32, isOutput=False)
    wv_d = nc.declare_dram_parameter("Wv", [D, D], F32, isOutput=False)
    bv_d = nc.declare_dram_parameter("bv", [D], F32, isOutput=False)
    o_d = nc.declare_dram_parameter("out", [S, D], F32, isOutput=True)
    dram = (x_d, m_d, wq_d, bq_d, wk_d, bk_d, wv_d, bv_d, o_d)

    with tile.TileContext(nc) as tc:
        with (
            tc.tile_pool(name="consts", bufs=1) as cst,
            tc.tile_pool(name="xT", bufs=1) as xT_pool,
            tc.tile_pool(name="qT", bufs=1) as qT_pool,
            tc.tile_pool(name="kT", bufs=1) as kT_pool,
            tc.tile_pool(name="vsb", bufs=1) as v_pool,
            tc.tile_pool(name="wx", bufs=8) as wx_pool,
            tc.tile_pool(name="pT", bufs=18) as p_pool,
            tc.tile_pool(name="small", bufs=6) as small_pool,
            # PSUM: transposes 2x1 banks, proj/scores [128,1024] 2x2 banks,
            # ctx 2x1 banks -> 8 banks total.
            tc.tile_pool(name="pst", bufs=2, space="PSUM") as ps_t,
            tc.tile_pool(name="psbig", bufs=2, space="PSUM") as ps_big,
            tc.tile_pool(name="psctx", bufs=2, space="PSUM") as ps_ctx,  # ctxT [65,512] 1 bank x2
        ):
            ident = cst.tile([P, P], F32, name="ident", tag="ident")
            make_identity(nc, ident)
            pools = (cst, xT_pool, qT_pool, kT_pool, v_pool, wx_pool, p_pool,
                     small_pool, ps_t, ps_big, ps_ctx, ident)
            if n_loop:
                with tc.For_i(0, n_loop, 1):
                    emit_body(nc, dram, pools)
            else:
                for _ in range(n_reps):
                    emit_body(nc, dram, pools)
    nc.compile()
    return nc


_NC_CACHE = None


def _get_nc():
    global _NC_CACHE
    if _NC_CACHE is None:
        _NC_CACHE = build_program()
    return _NC_CACHE


def make_in_maps(hidden_states, attention_mask, Wq, bq, Wk, bk, Wv, bv):
    hs = np.ascontiguousarray(np.asarray(hidden_states, dtype=np.float32))
    am = np.ascontiguousarray(
        np.asarray(attention_mask, dtype=np.float32).reshape(B, S)
    )
    shared = {
        "Wq": np.ascontiguousarray(np.asarray(Wq, dtype=np.float32)),
        "bq": np.ascontiguousarray(np.asarray(bq, dtype=np.float32)),
        "Wk": np.ascontiguousarray(np.asarray(Wk, dtype=np.float32)),
        "bk": np.ascontiguousarray(np.asarray(bk, dtype=np.float32)),
        "Wv": np.ascontiguousarray(np.asarray(Wv, dtype=np.float32)),
        "bv": np.ascontiguousarray(np.asarray(bv, dtype=np.float32)),
    }
    return [
        {"hidden_states": hs[b], "attention_mask": am[b], **shared}
        for b in range(B)
    ]


def kernel(hidden_states, attention_mask, Wq, bq, Wk, bk, Wv, bv):
    nc = _get_nc()
    in_maps = make_in_maps(hidden_states, attention_mask, Wq, bq, Wk, bk, Wv, bv)
    res = run_bass_kernel_spmd(nc, in_maps, list(range(N_CORES))).results
    out = np.stack([np.asarray(res[b]["out"], dtype=np.float32) for b in range(B)])
    return out


# revision 17
# speedup vs baseline: 1.0119x; 1.0119x over previous
"""BERT self-attention (B=8, S=1024, D=1024, H=16, DH=64) on 8 Trainium2 cores.

Strategy: pure data-parallel over batch - each of the 8 cores runs the full
self-attention for one batch element. No collectives.

Per-core kernel layout (S=seq, D=model, H=heads, DH=64):
  - X^T built once via PE transposes (fp32, 64 tiles of 128x128).
  - Q^T[j,s], K^T[j,s] computed directly in transposed orientation
    (contraction over d_in on partitions); biases folded in as K=1 rank-1
    matmuls (b x ones).  Each weight tile is double-pumped over both 512-col
    halves of a [128,1024] PSUM tile (consecutive same-weight matmuls skip
    the serial weight reload - measured 2.2x faster).
  - V[s,j] in natural orientation (lhsT = X^T as weights), stored bf16 in a
    head-interleaved layout of 65-column blocks: [64 V cols | ones col] per
    head.  The ones column makes the context matmul emit the softmax
    denominator for free.
  - scores computed TRANSPOSED: S^T[k,q], so the attention mask (indexed by
    k) is a per-partition bias folded with the 1/sqrt(DH) scale into the Exp
    activation: P^T = exp(scale*S^T + mask[k]), output bf16.
  - context: ctx[q,0:64] + rowsum at col 64 via lhsT=P^T tile (bf16),
    rhs = V' block [128,65]; normalize with vector reciprocal +
    per-partition tensor_scalar multiply, DMA straight to DRAM.
  - attention is software-pipelined by one head: PE runs ctx(h-1) while ACT
    runs exp(h), keeping both engines busy.
  - matmul dtypes: float32r for projections/scores; bf16 for probs@V.

Built on bacc.Bacc: its compile() legalizes sync waits (1 wait/instruction
hardware limit) via move_matmul_waits_to_ldweights + generate_event_semaphores.
"""

import numpy as np

import concourse.bass as bass
import concourse.bacc as bacc
import concourse.mybir as mybir
import concourse.tile as tile
from concourse.bass_utils import run_bass_kernel_spmd
from concourse.masks import make_identity

F32 = mybir.dt.float32
F32R = mybir.dt.float32r
BF16 = mybir.dt.bfloat16

B, S, D, H = 8, 1024, 1024, 16
DH = D // H  # 64
P = 128
NT = S // P  # 8 tiles along any 1024 dim
SC = S // 512  # 2 chunks of 512
SCALE = 1.0 / float(np.sqrt(DH))
N_CORES = 8
VW = DH + 1  # 65: V block width per head (64 cols + ones col)

PHASES = 7  # bitmask: 1=x^T, 2=projections, 4=attention (profiling aid)


def emit_body(nc, dram, pools):
    (x_d, m_d, wq_d, bq_d, wk_d, bk_d, wv_d, bv_d, o_d) = dram
    (cst, xT_pool, qT_pool, kT_pool, v_pool, wx_pool, p_pool, small_pool,
     ps_t, ps_big, ps_ctx, ident) = pools

    # ---- per-body constants (mask / bias rows) ----
    mask_cols = cst.tile([P, NT], F32, name="mask_cols", tag="mask_cols")
    nc.sync.dma_start(out=mask_cols, in_=m_d.ap().rearrange("(g p) -> p g", p=P))
    ones_f32 = cst.tile([1, 512], F32, name="ones_f32", tag="ones_f32")
    nc.vector.memset(ones_f32, 1.0)
    ones_row = cst.tile([1, 512], F32R, name="ones_row", tag="ones_row")
    nc.vector.tensor_copy(ones_row, ones_f32)
    b_rows = {}
    for nm, hd in (("bq", bq_d), ("bk", bk_d), ("bv", bv_d)):
        t = cst.tile([1, D], F32R, name=f"brow_{nm}", tag=f"brow_{nm}")
        nc.sync.dma_start(out=t, in_=hd.ap().unsqueeze(0).bitcast(F32R))
        b_rows[nm] = t

    if not PHASES & 1:
        return
    # ---- phase 1: X^T via PE transposes ----
    xT = []
    for it in range(NT):
        xT.append(xT_pool.tile([P, S], F32R, name=f"xT{it}", tag=f"xT{it}"))
    for st in range(NT):
        x_t = wx_pool.tile([P, D], F32, name="x_tile", tag="wx")
        nc.sync.dma_start(out=x_t, in_=x_d.ap()[st * P : (st + 1) * P, :])
        for it in range(NT):
            pt = ps_t.tile([P, P], F32, name="pt", tag="mm")
            nc.tensor.transpose(pt, x_t[:, it * P : (it + 1) * P], ident)
            nc.vector.tensor_copy(xT[it][:, st * P : (st + 1) * P], pt)

    if not PHASES & 2:
        fin = small_pool.tile([P, DH], F32, name="fin1", tag="bounce")
        nc.vector.tensor_copy(fin, xT[0][:, 0:DH].bitcast(F32))
        nc.sync.dma_start(out=o_d.ap()[0:P, 0:DH], in_=fin)
        return

    # ---- phase 2: projections (double-pumped weights) ----
    def load_w(w_d):
        tiles = []
        for it in range(NT):
            t = wx_pool.tile([P, D], F32R, name="w_tile", tag="wx")
            nc.sync.dma_start(
                out=t, in_=w_d.ap()[it * P : (it + 1) * P, :].bitcast(F32R)
            )
            tiles.append(t)
        return tiles

    # Q^T and K^T: out[j, s] = sum_i W[i, j] * X^T[i, s] + b[j]
    proj_T = {}
    for nm, w_dram, dst_pool in (("bq", wq_d, qT_pool), ("bk", wk_d, kT_pool)):
        w_tiles = load_w(w_dram)
        dst = []
        for jt in range(NT):
            dst.append(
                dst_pool.tile([P, S], F32R, name=f"{nm}T{jt}", tag=f"{nm}T{jt}")
            )
        for jt in range(NT):
            mm = ps_big.tile([P, S], F32, name="mm", tag="big")
            for it in range(NT):
                for sc in range(SC):
                    nc.tensor.matmul(
                        mm[:, sc * 512 : (sc + 1) * 512],
                        lhsT=w_tiles[it][:, jt * P : (jt + 1) * P],
                        rhs=xT[it][:, sc * 512 : (sc + 1) * 512],
                        start=(it == 0),
                        stop=False,
                    )
            for sc in range(SC):
                nc.tensor.matmul(
                    mm[:, sc * 512 : (sc + 1) * 512],
                    lhsT=b_rows[nm][0:1, jt * P : (jt + 1) * P],
                    rhs=ones_row,
                    start=False,
                    stop=True,
                )
            nc.vector.tensor_copy(dst[jt], mm)
        proj_T[nm] = dst
    qT, kT = proj_T["bq"], proj_T["bk"]

    # V: out[s, j] = sum_i X^T[i, s] * Wv[i, j] + bv[j], stored bf16 in
    # 65-wide head blocks with a trailing ones column.
    wv_tiles = load_w(wv_d)
    v_sb = []
    for st in range(NT):
        v = v_pool.tile([P, H * VW], BF16, name=f"v{st}", tag=f"v{st}")
        nc.gpsimd.memset(v, 1.0)  # ones columns survive at h*65+64
        v_sb.append(v)
    for st in range(NT):
        mm = ps_big.tile([P, S], F32, name="mmv", tag="big")
        for it in range(NT):
            for jc in range(SC):
                nc.tensor.matmul(
                    mm[:, jc * 512 : (jc + 1) * 512],
                    lhsT=xT[it][:, st * P : (st + 1) * P],
                    rhs=wv_tiles[it][:, jc * 512 : (jc + 1) * 512],
                    start=(it == 0),
                    stop=False,
                )
        for jc in range(SC):
            nc.tensor.matmul(
                mm[:, jc * 512 : (jc + 1) * 512],
                lhsT=ones_row[0:1, 0:P],
                rhs=b_rows["bv"][0:1, jc * 512 : (jc + 1) * 512],
                start=False,
                stop=True,
            )
        dst = v_sb[st].rearrange("p (g c) -> p g c", c=VW)[:, :, 0:DH]
        src = mm.rearrange("p (g c) -> p g c", c=DH)
        nc.vector.tensor_copy(dst, src)

    if not PHASES & 4:
        fin = small_pool.tile([P, DH], F32, name="fin2", tag="bounce")
        nc.vector.tensor_copy(fin, qT[0][:, 0:DH].bitcast(F32))
        nc.sync.dma_start(out=o_d.ap()[0:P, 0:DH], in_=fin)
        fin2 = small_pool.tile([P, DH], F32, name="fin3", tag="bounce")
        nc.vector.tensor_copy(fin2, kT[0][:, 0:DH].bitcast(F32))
        nc.sync.dma_start(out=o_d.ap()[0:P, DH : 2 * DH], in_=fin2)
        return

    # ---- phase 3: attention, software-pipelined by one head ----
    def emit_scores_exp(h):
        jt, ro = h // 2, (h % 2) * DH
        pT = []
        for kt in range(NT):
            sps = ps_big.tile([P, S], F32, name="sps", tag="big")
            for qc in range(SC):
                nc.tensor.matmul(
                    sps[:, qc * 512 : (qc + 1) * 512],
                    lhsT=kT[jt][ro : ro + DH, kt * P : (kt + 1) * P],
                    rhs=qT[jt][ro : ro + DH, qc * 512 : (qc + 1) * 512],
                    start=True,
                    stop=True,
                )
            pt = p_pool.tile([P, S], BF16, name="pT", tag="pT")
            nc.scalar.activation(
                pt,
                sps,
                mybir.ActivationFunctionType.Exp,
                bias=mask_cols[:, kt : kt + 1],
                scale=SCALE,
            )
            pT.append(pt)
        return pT

    def emit_ctx(h, pT):
        for qt in range(NT):
            cps = ps_ctx.tile([P, VW], F32, name="cps", tag="ctx")
            for kt in range(NT):
                nc.tensor.matmul(
                    cps,
                    lhsT=pT[kt][:, qt * P : (qt + 1) * P],
                    rhs=v_sb[kt][:, h * VW : (h + 1) * VW],
                    start=(kt == 0),
                    stop=(kt == NT - 1),
                )
            r = small_pool.tile([P, 1], F32, name="recip", tag="recip")
            nc.vector.reciprocal(r, cps[:, DH : DH + 1])
            bounce = small_pool.tile([P, DH], F32, name="bounce", tag="bounce")
            nc.vector.tensor_scalar_mul(bounce, cps[:, 0:DH], r)
            nc.sync.dma_start(
                out=o_d.ap()[qt * P : (qt + 1) * P, h * DH : (h + 1) * DH],
                in_=bounce,
            )

    prev = None
    for h in range(H):
        pT = emit_scores_exp(h)
        if prev is not None:
            emit_ctx(h - 1, prev)
        prev = pT
    emit_ctx(H - 1, prev)


def build_program(n_reps: int = 1, n_loop: int = 0) -> bass.Bass:
    nc = bacc.Bacc(trn_type="TRN2", target_bir_lowering=False, debug=False)

    x_d = nc.declare_dram_parameter("hidden_states", [S, D], F32, isOutput=False)
    m_d = nc.declare_dram_parameter("attention_mask", [S], F32, isOutput=False)
    wq_d = nc.declare_dram_parameter("Wq", [D, D], F32, isOutput=False)
    bq_d = nc.declare_dram_parameter("bq", [D], F32, isOutput=False)
    wk_d = nc.declare_dram_parameter("Wk", [D, D], F32, isOutput=False)
    bk_d = nc.declare_dram_parameter("bk", [D], F32, isOutput=False)
    wv_d = nc.declare_dram_parameter("Wv", [D, D], F32, isOutput=False)
    bv_d = nc.declare_dram_parameter("bv", [D], F32, isOutput=False)
    o_d = nc.declare_dram_parameter("out", [S, D], F32, isOutput=True)
    dram = (x_d, m_d, wq_d, bq_d, wk_d, bk_d, wv_d, bv_d, o_d)

    with tile.TileContext(nc) as tc:
        with (
            tc.tile_pool(name="consts", bufs=1) as cst,
            tc.tile_pool(name="xT", bufs=1) as xT_pool,
            tc.tile_pool(name="qT", bufs=1) as qT_pool,
            tc.tile_pool(name="kT", bufs=1) as kT_pool,
            tc.tile_pool(name="vsb", bufs=1) as v_pool,
            tc.tile_pool(name="wx", bufs=8) as wx_pool,
            tc.tile_pool(name="pT", bufs=16) as p_pool,
            tc.tile_pool(name="small", bufs=16) as small_pool,
            # PSUM: transposes 2x1 banks, proj/scores [128,1024] 2x2 banks,
            # ctx 2x1 banks -> 8 banks total.
            tc.tile_pool(name="pst", bufs=2, space="PSUM") as ps_t,
            tc.tile_pool(name="psbig", bufs=2, space="PSUM") as ps_big,
            tc.tile_pool(name="psctx", bufs=2, space="PSUM") as ps_ctx,  # ctxT [65,512] 1 bank x2
        ):
            ident = cst.tile([P, P], F32, name="ident", tag="ident")
            make_identity(nc, ident)
            pools = (cst, xT_pool, qT_pool, kT_pool, v_pool, wx_pool, p_pool,
                     small_pool, ps_t, ps_big, ps_ctx, ident)
            if n_loop:
                with tc.For_i(0, n_loop, 1):
                    emit_body(nc, dram, pools)
            else:
                for _ in range(n_reps):
                    emit_body(nc, dram, pools)
    nc.compile()
    return nc


_NC_CACHE = None


def _get_nc():
    global _NC_CACHE
    if _NC_CACHE is None:
        _NC_CACHE = build_program()
    return _NC_CACHE


def make_in_maps(hidden_states, attention_mask, Wq, bq, Wk, bk, Wv, bv):
    hs = np.ascontiguousarray(np.asarray(hidden_states, dtype=np.float32))
    am = np.ascontiguousarray(
        np.asarray(attention_mask, dtype=np.float32).reshape(B, S)
    )
    shared = {
        "Wq": np.ascontiguousarray(np.asarray(Wq, dtype=np.float32)),
        "bq": np.ascontiguousarray(np.asarray(bq, dtype=np.float32)),
        "Wk": np.ascontiguousarray(np.asarray(Wk, dtype=np.float32)),
        "bk": np.ascontiguousarray(np.asarray(bk, dtype=np.float32)),
        "Wv": np.ascontiguousarray(np.asarray(Wv, dtype=np.float32)),
        "bv": np.ascontiguousarray(np.asarray(bv, dtype=np.float32)),
    }
    return [
        {"hidden_states": hs[b], "attention_mask": am[b], **shared}
        for b in range(B)
    ]


def kernel(hidden_states, attention_mask, Wq, bq, Wk, bk, Wv, bv):
    nc = _get_nc()
    in_maps = make_in_maps(hidden_states, attention_mask, Wq, bq, Wk, bk, Wv, bv)
    res = run_bass_kernel_spmd(nc, in_maps, list(range(N_CORES))).results
    out = np.stack([np.asarray(res[b]["out"], dtype=np.float32) for b in range(B)])
    return out
